# revision 1
# baseline (speedup 1.0000x reference)
"""CRF sequence-score kernel for Trainium2 (8 NeuronCores, SPMD).

Strategy (S-shard: core k owns s in [64k, 64k+64), all 512 batches):
  rows r = s_local*512 + b, laid out as [q = r%128 partitions, x = r//128].
  - emit[r] = emissions[r, tags[r]] via one fused DVE scalar_tensor_tensor
    per 128-row block: accum_out = sum_t (iota_t == tag) * em[r, t].
  - trans[r] = T[tag_r, tagnext_r] via PE chain per block: broadcast-matmul
    of the block's tag row -> transposed one-hot (tensor_scalar vs partition
    iota) -> matmul fetches T rows into PSUM -> same stt selects tagnext.
  - masks folded in a small epilogue; reduction over s via AP-strided
    tensor_reduce; start/end terms via 1-idx-per-partition indirect DMA
    (end term computed exactly: mask column-sum -> last tag gather -> end
    table gather).
Host sums the 8 per-core [128, 4] partials; score[b] = total[b%128, b//128].
"""
import numpy as np

SEQ, BATCH, NTAGS = 512, 512, 128
NCORES = 8
SLICE = SEQ // NCORES            # 64 s-rows per core
NROWS = SLICE * BATCH            # 32768 rows per core
NBLK = NROWS // 128              # 256 blocks of 128 rows
P = 128

_RUNNER = None


# ---------------------------------------------------------------------------
# walrus workaround: this build allows only ONE sync-wait per instruction.
def _install_tile_patch():
    import bass_rust
    import concourse.mybir as mybir
    import concourse.tile as tile
    from concourse.vector_clock import ScopedClock

    if getattr(tile.TileContext, "_crf_patched", False):
        return

    def _drain_and_barrier(self, tick_clock, wait_clock):
        nc = self.nc
        drain_inst = nc.sync.drain()
        wait_clock.add_sem_waits(
            drain_inst.ins, ScopedClock({None: tick_clock.global_clock})
        )
        si = drain_inst.ins.sync_info
        waits = list(si.on_wait) if si is not None and si.on_wait else []
        if len(waits) > 1:
            si.on_wait = waits[:1]
            for w in waits[1:]:
                extra = nc.sync.drain()
                if extra.ins.sync_info is None:
                    extra.ins.sync_info = bass_rust.SyncInfo(on_wait=[], on_update=[])
                extra.ins.sync_info.on_wait = [w]
        nc.all_engine_barrier()
        assert self.sems is not None
        popped = nc._tile_sem_poison_stack.pop()
        assert popped is self._sem_poison
        nc.clear_and_free_semaphores(list(self.sems.allocated().values()))
        nc.all_engine_barrier()

    orig_commit = tile.TileContext._commit_instruction

    def _commit(self, inst, lazy_reg_writes=True):
        si = getattr(inst, "sync_info", None)
        if (
            si is not None
            and si.on_wait
            and len(si.on_wait) > 1
            and inst.engine != mybir.EngineType.Unassigned
        ):
            waits = list(si.on_wait)
            si.on_wait = waits[:1]
            for w in waits[1:]:
                nop = mybir.InstNoOp(name=f"I-{self.nc.next_id()}", ins=[], outs=[])
                nop.engine = inst.engine
                nop.sync_info = bass_rust.SyncInfo(on_wait=[w], on_update=[])
                self._add_instruction(nop)
        return orig_commit(self, inst, lazy_reg_writes)

    tile.TileContext._drain_and_barrier = _drain_and_barrier
    tile.TileContext._commit_instruction = _commit
    tile.TileContext._crf_patched = True


# ---------------------------------------------------------------------------
def _build_nc(skip_main=False, no_trans=False, phase_split=False):
    import concourse.bass as bass
    import concourse.mybir as mybir
    import concourse.tile as tile
    from concourse.masks import make_identity

    F32, I32, BF16, I16 = (mybir.dt.float32, mybir.dt.int32,
                           mybir.dt.bfloat16, mybir.dt.int16)
    AL = mybir.AluOpType

    nc = bass.Bass()
    em = nc.declare_dram_parameter("em", [NROWS * NTAGS], F32, isOutput=False)
    tagx_i = nc.declare_dram_parameter("tagx_i", [NROWS * 2], I32, isOutput=False)
    tagnx_i = nc.declare_dram_parameter("tagnx_i", [NROWS * 2], I32, isOutput=False)
    maskem_i = nc.declare_dram_parameter("maskem_i", [NROWS], I32, isOutput=False)
    masktr_i = nc.declare_dram_parameter("masktr_i", [NROWS], I32, isOutput=False)
    tmat_bf = nc.declare_dram_parameter("tmat_bf", [P, NTAGS], BF16, isOutput=False)
    startv = nc.declare_dram_parameter("startv", [NTAGS, 1], F32, isOutput=False)
    endv = nc.declare_dram_parameter("endv", [NTAGS, 1], F32, isOutput=False)
    maskf_i = nc.declare_dram_parameter("maskf_i", [SEQ * BATCH], I32, isOutput=False)
    tagf_i = nc.declare_dram_parameter("tagf_i", [SEQ * BATCH * 2, 1], I32, isOutput=False)
    out = nc.declare_dram_parameter("out", [P, 4], F32, isOutput=True)

    with tile.TileContext(nc) as tc:
        with tc.tile_pool(name="sbuf", bufs=1) as sb, \
             tc.tile_pool(name="psum", bufs=1, space="PSUM") as ps, \
             tc.tile_pool(name="emp", bufs=3) as emp:
            # ---- constants
            iota_i = sb.tile([P, NTAGS], I32, name="iota_i")
            nc.gpsimd.iota(iota_i[:], pattern=[[1, NTAGS]], base=0, channel_multiplier=0)
            iota = sb.tile([P, NTAGS], F32, name="iota")
            nc.vector.tensor_copy(out=iota[:], in_=iota_i[:])
            iop_i = sb.tile([P, 1], I32, name="iop_i")
            nc.gpsimd.iota(iop_i[:], pattern=[[0, 1]], base=0, channel_multiplier=1)
            iop = sb.tile([P, 1], F32, name="iop")
            nc.vector.tensor_copy(out=iop[:], in_=iop_i[:])
            niop = sb.tile([P, 1], F32, name="niop")
            nc.vector.tensor_scalar(out=niop[:], in0=iop[:], scalar1=-1.0,
                                    scalar2=None, op0=AL.mult)
            ones = sb.tile([P, P], F32, name="ones")
            nc.vector.memset(ones[:], 1.0)
            ident = sb.tile([P, P], F32, name="ident")
            make_identity(nc, ident[:])

            # ---- T matrix (bf16) stationary
            tmat = sb.tile([P, NTAGS], BF16, name="tmat")
            nc.sync.dma_start(out=tmat[:], in_=tmat_bf[:])

            # ---- tag/mask staging: x-major loads -> f32 -> PE transpose
            # TAGX[p, sub*128+m] = tag[128*(sub*128+p) + m]
            def stage_tags(name, dram):
                raw = sb.tile([P, 512], I32, name=f"{name}_raw")
                nc.sync.dma_start(
                    out=raw[:].rearrange("p (s i) -> p s i", s=2),
                    in_=dram[:].rearrange("(s p i) -> p s i", s=2, p=P, i=256),
                )
                f = sb.tile([P, 256], F32, name=f"{name}_f")
                nc.vector.tensor_copy(
                    out=f[:].rearrange("p (s m) -> p s m", s=2),
                    in_=raw[:].rearrange("p (s m two) -> p s m two", s=2, two=2)[:, :, :, 0:1],
                )
                return f

            def stage_mask(name, dram):
                raw = sb.tile([P, 256], I32, name=f"{name}_raw")
                nc.sync.dma_start(
                    out=raw[:].rearrange("p (s i) -> p s i", s=2),
                    in_=dram[:].rearrange("(s p i) -> p s i", s=2, p=P, i=P),
                )
                f = sb.tile([P, 256], F32, name=f"{name}_f")
                nc.vector.tensor_copy(out=f[:], in_=raw[:])
                return f

            tagx = stage_tags("tagx", tagx_i)     # [128, 2, 128] f32
            tagnx = stage_tags("tagnx", tagnx_i)
            mex = stage_mask("mex", maskem_i)
            mtx = stage_mask("mtx", masktr_i)

            # transpose halves -> [q, x] layout [128, 256]
            def transpose_qx(name, src):
                dst = sb.tile([P, 256], F32, name=f"{name}_t")
                for h in range(2):
                    tp = ps.tile([P, P], F32, name=f"{name}_tp{h}", tag=f"tb{h}")
                    nc.tensor.transpose(out=tp[:], in_=src[:, h * P:(h + 1) * P],
                                        identity=ident[:])
                    nc.scalar.copy(out=dst[:, h * P:(h + 1) * P], in_=tp[:])
                return dst

            tagt = transpose_qx("tagt", tagx)     # tag in [q, x]
            tagnt = transpose_qx("tagnt", tagnx)  # tagnext in [q, x]
            memt = transpose_qx("memt", mex)      # maskEM in [q, x]
            mtrt = transpose_qx("mtrt", mtx)      # maskTR in [q, x]

            # ---- start term: SV[q, j] = startv[tag[0, b]], b = 128j+q
            sidx = sb.tile([P, 4], I32, name="sidx")
            nc.vector.tensor_copy(out=sidx[:], in_=tagt[:, 0:4])
            sv = sb.tile([P, 4], F32, name="sv")
            for j in range(4):
                nc.gpsimd.indirect_dma_start(
                    out=sv[:, j:j + 1], out_offset=None, in_=startv[:],
                    in_offset=bass.IndirectOffsetOnAxis(ap=sidx[:, j:j + 1], axis=0),
                )

            # ---- end term (exact): mask col-sums -> last tag -> endv
            mf = sb.tile([P, 2048], I32, name="mf")
            nc.sync.dma_start(out=mf[:],
                              in_=maskf_i[:].rearrange("(p f) -> p f", p=P))
            ms1i = sb.tile([P, 512], I32, name="ms1i")
            with nc.allow_low_precision(reason="int32 mask counts are exact"):
                nc.vector.tensor_reduce(
                    out=ms1i[:],
                    in_=mf[:].rearrange("p (sl b) -> p b sl", b=512),
                    axis=mybir.AxisListType.X, op=AL.add,
                )
            ms1 = sb.tile([P, 512], F32, name="ms1")
            nc.vector.tensor_copy(out=ms1[:], in_=ms1i[:])
            msq = sb.tile([P, 4], F32, name="msq")
            for j in range(4):
                mp = ps.tile([P, 1], F32, name=f"mp{j}", tag="tb0")
                nc.tensor.matmul(out=mp[:], lhsT=ms1[:, j * P:(j + 1) * P],
                                 rhs=ones[:, 0:1], start=True, stop=True)
                nc.vector.tensor_copy(out=msq[:, j:j + 1], in_=mp[:])
            # si = ((msq - 1) * 512 + b) * 2,  b = 128j + q
            iop2_i = sb.tile([P, 1], I32, name="iop2_i")
            nc.gpsimd.iota(iop2_i[:], pattern=[[0, 1]], base=0, channel_multiplier=2)
            iop2 = sb.tile([P, 1], F32, name="iop2")
            nc.vector.tensor_copy(out=iop2[:], in_=iop2_i[:])
            j256_i = sb.tile([P, 4], I32, name="j256_i")
            nc.gpsimd.iota(j256_i[:], pattern=[[256, 4]], base=0, channel_multiplier=0)
            j256 = sb.tile([P, 4], F32, name="j256")
            nc.vector.tensor_copy(out=j256[:], in_=j256_i[:])
            si_f = sb.tile([P, 4], F32, name="si_f")
            nc.vector.tensor_scalar(out=si_f[:], in0=msq[:], scalar1=1024.0,
                                    scalar2=-1024.0, op0=AL.mult, op1=AL.add)
            nc.vector.tensor_scalar(out=si_f[:], in0=si_f[:], scalar1=iop2[:],
                                    scalar2=None, op0=AL.add)
            nc.vector.tensor_tensor(out=si_f[:], in0=si_f[:], in1=j256[:], op=AL.add)
            si4 = sb.tile([P, 4], I32, name="si4")
            nc.vector.tensor_copy(out=si4[:], in_=si_f[:])
            lt = sb.tile([P, 4], I32, name="lt")
            for j in range(4):
                nc.gpsimd.indirect_dma_start(
                    out=lt[:, j:j + 1], out_offset=None, in_=tagf_i[:],
                    in_offset=bass.IndirectOffsetOnAxis(ap=si4[:, j:j + 1], axis=0),
                )
            ev = sb.tile([P, 4], F32, name="ev")
            for j in range(4):
                nc.gpsimd.indirect_dma_start(
                    out=ev[:, j:j + 1], out_offset=None, in_=endv[:],
                    in_offset=bass.IndirectOffsetOnAxis(ap=lt[:, j:j + 1], axis=0),
                )

            # ---- main loop: emit-stt + trans chain per block x
            eacc = sb.tile([P, 256], F32, name="eacc")
            tacc = sb.tile([P, 256], F32, name="tacc")
            if skip_main:
                nc.vector.memset(eacc[:], 0.0)
            if skip_main or no_trans:
                nc.vector.memset(tacc[:], 0.0)
            junks = [sb.tile([P, NTAGS], F32, name=f"junk{i}", tag=f"jk{i}")
                     for i in range(16)]
            ohts = [sb.tile([P, P], BF16, name=f"oht{i}", tag=f"oh{i}")
                    for i in range(8)]
            d2s = [sb.tile([P, P], F32, name=f"d2_{i}", tag=f"d2{i}")
                   for i in range(8)]
            emch = None
            def do_emit(x):
                nonlocal emch
                d, sub = x // 16, x % 16
                if sub == 0:
                    emch = emp.tile([P, 16 * NTAGS], F32, name=f"emch{d}", tag="emch")
                    if d == 0:
                        for g in range(4):
                            nc.sync.dma_start(
                                out=emch[:].rearrange("p (s t) -> p s t", s=16)[:, g * 4:(g + 1) * 4, :],
                                in_=em[g * 4 * 16384:(g + 1) * 4 * 16384].rearrange(
                                    "(s p t) -> p s t", s=4, p=P, t=NTAGS),
                            )
                    else:
                        nc.sync.dma_start(
                            out=emch[:].rearrange("p (s t) -> p s t", s=16),
                            in_=em[d * 16 * 16384:(d + 1) * 16 * 16384].rearrange(
                                "(s p t) -> p s t", s=16, p=P, t=NTAGS),
                        )
                nc.vector.scalar_tensor_tensor(
                    out=junks[x % 16][:], in0=iota[:], scalar=tagt[:, x:x + 1],
                    in1=emch[:, sub * NTAGS:(sub + 1) * NTAGS], op0=AL.is_equal, op1=AL.mult,
                    accum_out=eacc[:, x:x + 1],
                )
            if phase_split and not skip_main:
                for x in range(NBLK):
                    do_emit(x)
            for x in range(0 if skip_main else NBLK):
                if not phase_split:
                    do_emit(x)
                # trans chain
                if no_trans:
                    continue
                tb = ps.tile([P, P], F32, name=f"tb{x % 4}", tag=f"tb{x % 4}")
                nc.tensor.transpose(out=tb[:],
                                    in_=tagt[:, x:x + 1].to_broadcast([P, P]),
                                    identity=ident[:])
                oht = ohts[x % 8]
                d2 = d2s[x % 8]
                nc.scalar.activation(out=d2[:], in_=tb[:],
                                     func=mybir.ActivationFunctionType.Square,
                                     bias=niop[:], scale=1.0)
                nc.scalar.activation(out=oht[:], in_=d2[:],
                                     func=mybir.ActivationFunctionType.Relu,
                                     bias=1.0, scale=-1.0)
                fp = ps.tile([P, P], F32, name=f"fp{x % 4}", tag=f"fp{x % 4}")
                nc.tensor.matmul(out=fp[:], lhsT=oht[:], rhs=tmat[:],
                                 start=True, stop=True)
                nc.vector.scalar_tensor_tensor(
                    out=junks[(x + 8) % 16][:], in0=iota[:], scalar=tagnt[:, x:x + 1],
                    in1=fp[:], op0=AL.is_equal, op1=AL.mult,
                    accum_out=tacc[:, x:x + 1],
                )

            # ---- epilogue: contrib = eacc*memt + tacc*mtrt, reduce over x//4
            c1 = sb.tile([P, 256], F32, name="c1")
            c2 = sb.tile([P, 256], F32, name="c2")
            cs = sb.tile([P, 256], F32, name="cs")
            for h in range(2):
                hs = slice(h * 128, (h + 1) * 128)
                nc.vector.tensor_tensor(out=c1[:, hs], in0=eacc[:, hs],
                                        in1=memt[:, hs], op=AL.mult)
                nc.vector.tensor_tensor(out=c2[:, hs], in0=tacc[:, hs],
                                        in1=mtrt[:, hs], op=AL.mult)
                nc.vector.tensor_tensor(out=cs[:, hs], in0=c1[:, hs],
                                        in1=c2[:, hs], op=AL.add)
            part = sb.tile([P, 4], F32, name="part")
            nc.vector.tensor_reduce(
                out=part[:],
                in_=cs[:].rearrange("p (u t) -> p t u", t=4),
                axis=mybir.AxisListType.X, op=AL.add,
            )

            # ---- total
            score = sb.tile([P, 4], F32, name="score")
            nc.vector.tensor_tensor(out=score[:], in0=part[:], in1=sv[:], op=AL.add)
            nc.vector.tensor_tensor(out=score[:], in0=score[:], in1=ev[:], op=AL.add)
            nc.sync.dma_start(out=out[:], in_=score[:])

    return nc


# ---------------------------------------------------------------------------
def _make_runner(nc, n_cores=8):
    import jax
    from jax.sharding import Mesh, PartitionSpec
    from jax.experimental.shard_map import shard_map
    import concourse.mybir as mybir
    from concourse import bass2jax

    bass2jax.install_neuronx_cc_hook()
    partition_name = nc.partition_id_tensor.name if nc.partition_id_tensor else None
    in_names, out_names, out_avals, zero_outs = [], [], [], []
    for alloc in nc.m.functions[0].allocations:
        if not isinstance(alloc, mybir.MemoryLocationSet):
            continue
        name = alloc.memorylocations[0].name
        if alloc.kind == "ExternalInput":
            if name != partition_name:
                in_names.append(name)
        elif alloc.kind == "ExternalOutput":
            shape = tuple(alloc.tensor_shape)
            dtype = mybir.dt.np(alloc.dtype)
            out_names.append(name)
            out_avals.append(jax.core.ShapedArray(shape, dtype))
            zero_outs.append(np.zeros(shape, dtype))
    n_params = len(in_names)
    all_in_names = list(in_names) + list(out_names)
    if partition_name is not None:
        all_in_names.append(partition_name)

    def _body(*args):
        operands = list(args)
        if partition_name is not None:
            operands.append(bass2jax.partition_id_tensor())
        outs = bass2jax._bass_exec_p.bind(
            *operands, out_avals=tuple(out_avals), in_names=tuple(all_in_names),
            out_names=tuple(out_names), lowering_input_output_aliases=(),
            sim_require_finite=True, sim_require_nnan=True, nc=nc,
        )
        return tuple(outs)

    devices = jax.devices()[:n_cores]
    mesh = Mesh(np.asarray(devices), ("core",))
    n_outs = len(out_names)
    jitted = jax.jit(
        shard_map(_body, mesh=mesh,
                  in_specs=(PartitionSpec("core"),) * (n_params + n_outs),
                  out_specs=(PartitionSpec("core"),) * n_outs, check_rep=False),
        keep_unused=True,
    )

    def run(in_maps):
        per_core = [[np.asarray(m[nm]) for nm in in_names] for m in in_maps]
        concat_in = [np.concatenate([per_core[c][i] for c in range(n_cores)], axis=0)
                     for i in range(n_params)]
        concat_zero = [np.concatenate([z] * n_cores, axis=0) for z in zero_outs]
        outs = [np.asarray(o) for o in jitted(*concat_in, *concat_zero)]
        results = []
        for c in range(n_cores):
            d = {}
            for i, nm in enumerate(out_names):
                per = outs[i].shape[0] // n_cores
                d[nm] = outs[i][c * per:(c + 1) * per]
            results.append(d)
        return results

    return run


def _get_runner():
    global _RUNNER
    if _RUNNER is None:
        _install_tile_patch()
        _RUNNER = _make_runner(_build_nc(), NCORES)
    return _RUNNER


# ---------------------------------------------------------------------------
def make_in_maps(emissions, tags, mask, start_transitions, end_transitions,
                 transitions):
    import ml_dtypes

    emissions = np.ascontiguousarray(emissions, dtype=np.float32)
    tags = np.ascontiguousarray(tags, dtype=np.int64)
    mask = np.ascontiguousarray(mask, dtype=np.int32)
    tmat_bf = np.ascontiguousarray(
        transitions.astype(ml_dtypes.bfloat16))
    startv = np.ascontiguousarray(start_transitions, np.float32).reshape(NTAGS, 1)
    endv = np.ascontiguousarray(end_transitions, np.float32).reshape(NTAGS, 1)
    maskf_i = np.ascontiguousarray(mask, np.int32).reshape(-1)
    tagf_i = tags.view(np.int32).reshape(-1, 1).copy()

    in_maps = []
    for k in range(NCORES):
        s0 = k * SLICE
        em_k = emissions[s0:s0 + SLICE].reshape(-1)
        tag_k = np.ascontiguousarray(tags[s0:s0 + SLICE]).view(np.int32).reshape(-1)
        if k < NCORES - 1:
            tagn_k = np.ascontiguousarray(tags[s0 + 1:s0 + SLICE + 1]).view(np.int32).reshape(-1)
            masktr_k = np.ascontiguousarray(mask[s0 + 1:s0 + SLICE + 1]).reshape(-1)
        else:
            tagn_k = np.ascontiguousarray(
                np.concatenate([tags[s0 + 1:], tags[-1:]])).view(np.int32).reshape(-1)
            masktr_k = np.concatenate(
                [mask[s0 + 1:], np.zeros((1, BATCH), np.int32)]).reshape(-1)
        maskem_k = mask[s0:s0 + SLICE].copy()
        if k == 0:
            maskem_k[0, :] = 1
        zero128 = np.zeros((NTAGS, 1), np.float32)
        in_maps.append({
            "em": em_k,
            "tagx_i": tag_k,
            "tagnx_i": tagn_k,
            "maskem_i": maskem_k.reshape(-1),
            "masktr_i": np.ascontiguousarray(masktr_k, np.int32),
            "tmat_bf": tmat_bf,
            "startv": startv if k == 0 else zero128,
            "endv": endv if k == NCORES - 1 else zero128,
            "maskf_i": maskf_i,
            "tagf_i": tagf_i,
        })
    return in_maps


def kernel(emissions, tags, mask, start_transitions, end_transitions,
           transitions):
    run = _get_runner()
    in_maps = make_in_maps(emissions, tags, mask, start_transitions,
                           end_transitions, transitions)
    results = run(in_maps)
    total = np.zeros((P, 4), np.float64)
    for r in results:
        total += r["out"].astype(np.float64)
    score = total.T.reshape(BATCH).astype(np.float32)
    return score



# revision 7
# speedup vs baseline: 1.1011x; 1.1011x over previous
"""CRF sequence-score kernel for Trainium2 (8 NeuronCores, SPMD).

Strategy (S-shard: core k owns s in [64k, 64k+64), all 512 batches):
  rows r = q*256 + x laid out as [q=128 partitions, x=256 cols];
  (s_local, b) = (q//2, 256*(q&1) + x).
  - em streamed bf16 in 16-block chunks (4KB descriptors, no small-desc
    penalty); emit[r] via DVE scalar_tensor_tensor in 4x bf16 mode.
  - trans[r] = T[tag_r, tagnext_r]: per block x, PE matmul
    lhsT=onehot(tagnext)*masktr (host-built fp8) x rhs=T^T (fp8 static)
    -> TN[m, t] = T[t, tagnext_m] in PSUM; copied PSUM->SBUF bf16 in
    4-block groups (alternating Act/DVE); then the same stt selects
    t = tag_m. masktr is folded into the one-hots by the host.
  - contrib = eacc*maskem + tacc; per-b sum over s via parity matmul
    (lhsT = [q&1 == h] one-hot) -> [2, 256] PSUM -> out.
  - start/end terms: baseline's exact indirect-DMA chain (mask col-sums ->
    last tag -> endv; startv via tag[0]) -> out2 [128, 4].
Host sums per-core outputs; score[b] = main[b//256, b%256] + se[b%128, b//128].
"""
import numpy as np

SEQ, BATCH, NTAGS = 512, 512, 128
NCORES = 8
SLICE = SEQ // NCORES            # 64 s-rows per core
NROWS = SLICE * BATCH            # 32768 rows per core
NBLK = NROWS // 128              # 256 blocks of 128 rows
P = 128
XC = 16                          # blocks per emissions chunk

_RUNNER = None


# ---------------------------------------------------------------------------
# walrus workaround: this build allows only ONE sync-wait per instruction.
def _install_tile_patch():
    import bass_rust
    import concourse.mybir as mybir
    import concourse.tile as tile
    from concourse.vector_clock import ScopedClock

    if getattr(tile.TileContext, "_crf_patched", False):
        return

    def _drain_and_barrier(self, tick_clock, wait_clock):
        nc = self.nc
        drain_inst = nc.sync.drain()
        wait_clock.add_sem_waits(
            drain_inst.ins, ScopedClock({None: tick_clock.global_clock})
        )
        si = drain_inst.ins.sync_info
        waits = list(si.on_wait) if si is not None and si.on_wait else []
        if len(waits) > 1:
            si.on_wait = waits[:1]
            for w in waits[1:]:
                extra = nc.sync.drain()
                if extra.ins.sync_info is None:
                    extra.ins.sync_info = bass_rust.SyncInfo(on_wait=[], on_update=[])
                extra.ins.sync_info.on_wait = [w]
        nc.all_engine_barrier()
        assert self.sems is not None
        popped = nc._tile_sem_poison_stack.pop()
        assert popped is self._sem_poison
        nc.clear_and_free_semaphores(list(self.sems.allocated().values()))
        nc.all_engine_barrier()

    orig_commit = tile.TileContext._commit_instruction

    def _commit(self, inst, lazy_reg_writes=True):
        si = getattr(inst, "sync_info", None)
        if (
            si is not None
            and si.on_wait
            and len(si.on_wait) > 1
            and inst.engine != mybir.EngineType.Unassigned
        ):
            waits = list(si.on_wait)
            si.on_wait = waits[:1]
            for w in waits[1:]:
                nop = mybir.InstNoOp(name=f"I-{self.nc.next_id()}", ins=[], outs=[])
                nop.engine = inst.engine
                nop.sync_info = bass_rust.SyncInfo(on_wait=[w], on_update=[])
                self._add_instruction(nop)
        return orig_commit(self, inst, lazy_reg_writes)

    tile.TileContext._drain_and_barrier = _drain_and_barrier
    tile.TileContext._commit_instruction = _commit
    tile.TileContext._crf_patched = True


# ---------------------------------------------------------------------------
def _build_nc():
    import concourse.bass as bass
    import concourse.mybir as mybir
    import concourse.tile as tile

    F32, I32, BF16 = mybir.dt.float32, mybir.dt.int32, mybir.dt.bfloat16
    FP8 = mybir.dt.float8e4
    AL = mybir.AluOpType

    nc = bass.Bass()
    em = nc.declare_dram_parameter("em", [NROWS * NTAGS], BF16, isOutput=False)
    oht_d = nc.declare_dram_parameter("oht", [P * NROWS], FP8, isOutput=False)
    ttab_d = nc.declare_dram_parameter("ttab", [P * NTAGS], FP8, isOutput=False)
    tagt_d = nc.declare_dram_parameter("tagt", [P * NBLK], BF16, isOutput=False)
    mem_d = nc.declare_dram_parameter("memf", [P * NBLK], F32, isOutput=False)
    par_d = nc.declare_dram_parameter("par", [P * 2], F32, isOutput=False)
    startv = nc.declare_dram_parameter("startv", [NTAGS, 1], F32, isOutput=False)
    endv = nc.declare_dram_parameter("endv", [NTAGS, 1], F32, isOutput=False)
    maskf_i = nc.declare_dram_parameter("maskf_i", [SEQ * BATCH], I32, isOutput=False)
    tagf_i = nc.declare_dram_parameter("tagf_i", [SEQ * BATCH * 2, 1], I32, isOutput=False)
    out_m = nc.declare_dram_parameter("out_m", [2, NBLK], F32, isOutput=True)
    out_se = nc.declare_dram_parameter("out_se", [P, 4], F32, isOutput=True)

    with tile.TileContext(nc) as tc:
        with tc.tile_pool(name="sbuf", bufs=1) as sb, \
             tc.tile_pool(name="psum", bufs=1, space="PSUM") as ps, \
             tc.tile_pool(name="emp", bufs=3) as emp:
            # ---- constants / staging
            iota_i = sb.tile([P, NTAGS], I32, name="iota_i")
            nc.gpsimd.iota(iota_i[:], pattern=[[1, NTAGS]], base=0, channel_multiplier=0)
            iota = sb.tile([P, NTAGS], BF16, name="iota")
            nc.vector.tensor_copy(out=iota[:], in_=iota_i[:])

            ttab = sb.tile([P, NTAGS], FP8, name="ttab")
            nc.sync.dma_start(out=ttab[:], in_=ttab_d[:].rearrange("(p t) -> p t", p=P))
            ohts = sb.tile([P, NROWS], FP8, name="ohts")
            nc.sync.dma_start(out=ohts[:], in_=oht_d[:].rearrange("(p r) -> p r", p=P))
            tagt = sb.tile([P, NBLK], BF16, name="tagt")
            nc.sync.dma_start(out=tagt[:], in_=tagt_d[:].rearrange("(p x) -> p x", p=P))
            memf = sb.tile([P, NBLK], F32, name="memf")
            nc.sync.dma_start(out=memf[:], in_=mem_d[:].rearrange("(p x) -> p x", p=P))
            par = sb.tile([P, 2], F32, name="par")
            nc.sync.dma_start(out=par[:], in_=par_d[:].rearrange("(p h) -> p h", p=P))

            # ---- start term: SV[q, j] = startv[tagq0[q, j]], b = 128j+q
            # tagq0 host-staged as cols 0:4 of tagt?  No: tags[0, b] in (q, j)
            # layout rides in tagf gather below; simplest: small gathers like
            # the baseline, with sidx staged via tagf_i[b*2] values.
            iop2_i = sb.tile([P, 1], I32, name="iop2_i")
            nc.gpsimd.iota(iop2_i[:], pattern=[[0, 1]], base=0, channel_multiplier=2)
            iop2 = sb.tile([P, 1], F32, name="iop2")
            nc.vector.tensor_copy(out=iop2[:], in_=iop2_i[:])
            j256_i = sb.tile([P, 4], I32, name="j256_i")
            nc.gpsimd.iota(j256_i[:], pattern=[[256, 4]], base=0, channel_multiplier=0)
            j256 = sb.tile([P, 4], F32, name="j256")
            nc.vector.tensor_copy(out=j256[:], in_=j256_i[:])
            # s0idx = (0*512 + b)*2 = 2b = iop2 + 256*j
            s0f = sb.tile([P, 4], F32, name="s0f")
            nc.vector.tensor_scalar(out=s0f[:], in0=j256[:], scalar1=iop2[:],
                                    scalar2=None, op0=AL.add)
            s0i = sb.tile([P, 4], I32, name="s0i")
            nc.vector.tensor_copy(out=s0i[:], in_=s0f[:])
            t0 = sb.tile([P, 4], I32, name="t0")
            for j in range(4):
                nc.gpsimd.indirect_dma_start(
                    out=t0[:, j:j + 1], out_offset=None, in_=tagf_i[:],
                    in_offset=bass.IndirectOffsetOnAxis(ap=s0i[:, j:j + 1], axis=0),
                )
            sv = sb.tile([P, 4], F32, name="sv")
            for j in range(4):
                nc.gpsimd.indirect_dma_start(
                    out=sv[:, j:j + 1], out_offset=None, in_=startv[:],
                    in_offset=bass.IndirectOffsetOnAxis(ap=t0[:, j:j + 1], axis=0),
                )

            # ---- end term (exact): mask col-sums -> last tag -> endv
            mf = sb.tile([P, 2048], I32, name="mf")
            nc.sync.dma_start(out=mf[:],
                              in_=maskf_i[:].rearrange("(p f) -> p f", p=P))
            ms1i = sb.tile([P, 512], I32, name="ms1i")
            with nc.allow_low_precision(reason="int32 mask counts are exact"):
                nc.vector.tensor_reduce(
                    out=ms1i[:],
                    in_=mf[:].rearrange("p (sl b) -> p b sl", b=512),
                    axis=mybir.AxisListType.X, op=AL.add,
                )
            ms1 = sb.tile([P, 512], F32, name="ms1")
            nc.vector.tensor_copy(out=ms1[:], in_=ms1i[:])
            ones = sb.tile([P, 1], F32, name="ones")
            nc.vector.memset(ones[:], 1.0)
            msq = sb.tile([P, 4], F32, name="msq")
            for j in range(4):
                mp = ps.tile([P, 1], F32, name=f"mp{j}", tag="pend")
                nc.tensor.matmul(out=mp[:], lhsT=ms1[:, j * P:(j + 1) * P],
                                 rhs=ones[:], start=True, stop=True)
                nc.vector.tensor_copy(out=msq[:, j:j + 1], in_=mp[:])
            # si = ((msq - 1) * 512 + b) * 2,  b = 128j + q
            si_f = sb.tile([P, 4], F32, name="si_f")
            nc.vector.tensor_scalar(out=si_f[:], in0=msq[:], scalar1=1024.0,
                                    scalar2=-1024.0, op0=AL.mult, op1=AL.add)
            nc.vector.tensor_scalar(out=si_f[:], in0=si_f[:], scalar1=iop2[:],
                                    scalar2=None, op0=AL.add)
            nc.vector.tensor_tensor(out=si_f[:], in0=si_f[:], in1=j256[:], op=AL.add)
            si4 = sb.tile([P, 4], I32, name="si4")
            nc.vector.tensor_copy(out=si4[:], in_=si_f[:])
            lt = sb.tile([P, 4], I32, name="lt")
            for j in range(4):
                nc.gpsimd.indirect_dma_start(
                    out=lt[:, j:j + 1], out_offset=None, in_=tagf_i[:],
                    in_offset=bass.IndirectOffsetOnAxis(ap=si4[:, j:j + 1], axis=0),
                )
            ev = sb.tile([P, 4], F32, name="ev")
            for j in range(4):
                nc.gpsimd.indirect_dma_start(
                    out=ev[:, j:j + 1], out_offset=None, in_=endv[:],
                    in_offset=bass.IndirectOffsetOnAxis(ap=lt[:, j:j + 1], axis=0),
                )
            sev = sb.tile([P, 4], F32, name="sev")
            nc.vector.tensor_tensor(out=sev[:], in0=sv[:], in1=ev[:], op=AL.add)
            nc.sync.dma_start(out=out_se[:], in_=sev[:])

            # ---- main loop: emit stt + trans (PE matmul -> copy -> stt)
            eacc = sb.tile([P, NBLK], F32, name="eacc")
            tacc = sb.tile([P, NBLK], F32, name="tacc")
            junks = [sb.tile([P, NTAGS], BF16, name=f"junk{i}", tag=f"jk{i}")
                     for i in range(16)]
            tnsbs = [sb.tile([P, 4 * NTAGS], BF16, name=f"tnsb{i}", tag=f"tn{i}")
                     for i in range(4)]
            emch = None
            for x in range(NBLK):
                d, sub = x // XC, x % XC
                if sub == 0:
                    emch = emp.tile([P, XC * NTAGS], BF16, name=f"emch{d}", tag="emch")
                    nc.sync.dma_start(
                        out=emch[:],
                        in_=em[:].rearrange("(q x t) -> q (x t)", q=P, x=NBLK)
                            [:, d * XC * NTAGS:(d + 1) * XC * NTAGS],
                    )
                nc.vector.scalar_tensor_tensor(
                    out=junks[x % 8][:], in0=iota[:], scalar=tagt[:, x:x + 1],
                    in1=emch[:, sub * NTAGS:(sub + 1) * NTAGS],
                    op0=AL.is_equal, op1=AL.mult,
                    accum_out=eacc[:, x:x + 1],
                )
                # trans: TN[m, t] = sum_p oht[p, m] * T[t, p]
                g, off = x // 4, (x % 4) * NTAGS
                if x % 4 == 0:
                    tn_ps = ps.tile([P, 4 * NTAGS], F32, name=f"tnps{g % 2}",
                                    tag=f"tb{g % 2}")
                nc.tensor.matmul(
                    out=tn_ps[:, off:off + NTAGS],
                    lhsT=ohts[:, x * P:(x + 1) * P],
                    rhs=ttab[:], start=True, stop=True,
                )
                if x % 4 == 3:
                    tnsb = tnsbs[g % 4]
                    if g % 2 == 0:
                        nc.scalar.copy(out=tnsb[:], in_=tn_ps[:])
                    else:
                        nc.vector.tensor_copy(out=tnsb[:], in_=tn_ps[:])
                    for xx in range(g * 4, g * 4 + 4):
                        o2 = (xx % 4) * NTAGS
                        nc.vector.scalar_tensor_tensor(
                            out=junks[8 + xx % 8][:], in0=iota[:],
                            scalar=tagt[:, xx:xx + 1],
                            in1=tnsb[:, o2:o2 + NTAGS],
                            op0=AL.is_equal, op1=AL.mult,
                            accum_out=tacc[:, xx:xx + 1],
                        )

            # ---- epilogue: contrib = eacc*memf + tacc; parity-sum over q
            contrib = sb.tile([P, NBLK], F32, name="contrib")
            nc.vector.tensor_tensor(out=contrib[:], in0=eacc[:], in1=memf[:],
                                    op=AL.mult)
            nc.vector.tensor_tensor(out=contrib[:], in0=contrib[:], in1=tacc[:],
                                    op=AL.add)
            mainp = ps.tile([2, NBLK], F32, name="mainp", tag="pend")
            nc.tensor.matmul(out=mainp[:], lhsT=par[:], rhs=contrib[:],
                             start=True, stop=True)
            mains = sb.tile([2, NBLK], F32, name="mains")
            nc.vector.tensor_copy(out=mains[:], in_=mainp[:])
            nc.sync.dma_start(out=out_m[:], in_=mains[:])

    return nc


# ---------------------------------------------------------------------------
def _make_runner(nc, n_cores=8):
    import jax
    from jax.sharding import Mesh, PartitionSpec
    from jax.experimental.shard_map import shard_map
    import concourse.mybir as mybir
    from concourse import bass2jax

    bass2jax.install_neuronx_cc_hook()
    partition_name = nc.partition_id_tensor.name if nc.partition_id_tensor else None
    in_names, out_names, out_avals, zero_outs = [], [], [], []
    for alloc in nc.m.functions[0].allocations:
        if not isinstance(alloc, mybir.MemoryLocationSet):
            continue
        name = alloc.memorylocations[0].name
        if alloc.kind == "ExternalInput":
            if name != partition_name:
                in_names.append(name)
        elif alloc.kind == "ExternalOutput":
            shape = tuple(alloc.tensor_shape)
            dtype = mybir.dt.np(alloc.dtype)
            out_names.append(name)
            out_avals.append(jax.core.ShapedArray(shape, dtype))
            zero_outs.append(np.zeros(shape, dtype))
    n_params = len(in_names)
    all_in_names = list(in_names) + list(out_names)
    if partition_name is not None:
        all_in_names.append(partition_name)

    def _body(*args):
        operands = list(args)
        if partition_name is not None:
            operands.append(bass2jax.partition_id_tensor())
        outs = bass2jax._bass_exec_p.bind(
            *operands, out_avals=tuple(out_avals), in_names=tuple(all_in_names),
            out_names=tuple(out_names), lowering_input_output_aliases=(),
            sim_require_finite=True, sim_require_nnan=True, nc=nc,
        )
        return tuple(outs)

    devices = jax.devices()[:n_cores]
    mesh = Mesh(np.asarray(devices), ("core",))
    n_outs = len(out_names)
    jitted = jax.jit(
        shard_map(_body, mesh=mesh,
                  in_specs=(PartitionSpec("core"),) * (n_params + n_outs),
                  out_specs=(PartitionSpec("core"),) * n_outs, check_rep=False),
        keep_unused=True,
    )

    def run(in_maps):
        per_core = [[np.asarray(m[nm]) for nm in in_names] for m in in_maps]
        concat_in = [np.concatenate([per_core[c][i] for c in range(n_cores)], axis=0)
                     for i in range(n_params)]
        concat_zero = [np.concatenate([z] * n_cores, axis=0) for z in zero_outs]
        outs = [np.asarray(o) for o in jitted(*concat_in, *concat_zero)]
        results = []
        for c in range(n_cores):
            d = {}
            for i, nm in enumerate(out_names):
                per = outs[i].shape[0] // n_cores
                d[nm] = outs[i][c * per:(c + 1) * per]
            results.append(d)
        return results

    return run


def _get_runner():
    global _RUNNER
    if _RUNNER is None:
        _install_tile_patch()
        _RUNNER = _make_runner(_build_nc(), NCORES)
    return _RUNNER


# ---------------------------------------------------------------------------
def make_in_maps(emissions, tags, mask, start_transitions, end_transitions,
                 transitions):
    import ml_dtypes
    BF16, FP8 = ml_dtypes.bfloat16, ml_dtypes.float8_e4m3

    emissions = np.asarray(emissions, dtype=np.float32)
    tg = np.asarray(tags).astype(np.int32)
    msk = np.asarray(mask).astype(np.int32)

    ttab = np.ascontiguousarray(np.asarray(transitions, np.float32).T).astype(FP8)
    startv = np.ascontiguousarray(start_transitions, np.float32).reshape(NTAGS, 1)
    endv = np.ascontiguousarray(end_transitions, np.float32).reshape(NTAGS, 1)
    maskf_i = np.ascontiguousarray(msk, np.int32).reshape(-1)
    tagf_i = tg.astype(np.int64).view(np.int32).reshape(-1, 1).copy()
    zero128 = np.zeros((NTAGS, 1), np.float32)
    par = np.zeros((P, 2), np.float32)
    par[0::2, 0] = 1.0
    par[1::2, 1] = 1.0

    # (q, x) grids: s_local = q//2, b = 256*(q&1) + x
    q = np.arange(P)[:, None]
    x = np.arange(NBLK)[None, :]
    slg = q // 2                              # [128, 1]
    bg = 256 * (q & 1) + x                    # [128, 256]

    in_maps = []
    for k in range(NCORES):
        s0 = k * SLICE
        sg = s0 + slg                          # global s, [128, 256] broadcast
        sgb = np.broadcast_to(sg, (P, NBLK))
        tag = tg[sgb, bg]                      # [128, 256]
        em_k = emissions[s0:s0 + SLICE].astype(BF16).reshape(-1)

        maskem = (msk[sgb, bg] != 0).astype(np.float32)
        if k == 0:
            maskem[sgb == 0] = 1.0  # reference counts emit[0] unconditionally

        # trans one-hots: oht[p, x*128 + q] = (tagnext[q,x]==p) * masktr[q,x]
        last = sgb == SEQ - 1
        snext = np.minimum(sgb + 1, SEQ - 1)
        masktr = np.where(last, 0, msk[snext, bg]).astype(np.float32)
        tagn = np.where(last, 0, tg[snext, bg])
        # build [128p, NROWS] fp8: col index = x*128 + q
        col = (x * P + q)                      # [128, 256] col for (q, x)
        oht = np.zeros((P, NROWS), dtype=np.float32)
        oht[tagn.reshape(-1), col.reshape(-1)] = masktr.reshape(-1)
        oht8 = oht.astype(FP8)

        in_maps.append({
            "em": em_k,
            "oht": oht8.reshape(-1),
            "ttab": ttab.reshape(-1),
            "tagt": tag.astype(BF16).reshape(-1),
            "memf": maskem.reshape(-1),
            "par": par.reshape(-1),
            "startv": startv if k == 0 else zero128,
            "endv": endv if k == NCORES - 1 else zero128,
            "maskf_i": maskf_i,
            "tagf_i": tagf_i,
        })
    return in_maps


def kernel(emissions, tags, mask, start_transitions, end_transitions,
           transitions):
    run = _get_runner()
    in_maps = make_in_maps(emissions, tags, mask, start_transitions,
                           end_transitions, transitions)
    results = run(in_maps)
    main = np.zeros((2, NBLK), np.float64)
    se = np.zeros((P, 4), np.float64)
    for r_ in results:
        main += r_["out_m"].astype(np.float64)
        se += r_["out_se"].astype(np.float64)
    score = main.reshape(BATCH)                      # b = h*256 + x
    b = np.arange(BATCH)
    score = score + se[b % P, b // P]                # se layout: b = j*128 + q
    return score.astype(np.float32)


# revision 9
# speedup vs baseline: 2.2013x; 1.9991x over previous
"""CRF sequence-score kernel for Trainium2 (8 NeuronCores, SPMD).

Strategy (S-shard: core k owns s in [64k, 64k+64), all 512 batches):
  rows r = q*256 + x laid out as [q=128 partitions, x=256 cols];
  (s_local, b) = (q//2, 256*(q&1) + x).
  Per block x, ONE PSUM tile accumulates  M[m, t] = em[row_m, t]
  + T[t, tagnext_m]*masktr_m  via two PE matmuls:
    (1) lhsT = host-built fp8 onehot(tagnext)*masktr,  rhs = T^T fp8
    (2) lhsT = identity bf16,                          rhs = em chunk bf16
  PSUM -> SBUF bf16 in 4-block groups (Act), then ONE DVE
  scalar_tensor_tensor per block selects t = tag_m:
    macc[m, x] = em[row, tag] + T[tag, tagnext]*masktr.
  contrib = macc * maskem  (exact for step masks: maskem=0 => masktr=0);
  per-b reduction over s via parity matmul -> [2, 256].
  start/end terms: TWO matmuls  startv/endv (fp8 col) x host one-hot of
  tag[0,b] / tags[seq_end,b]  -> [1, 512]  (start real on core 0 only,
  end on core 7 only).
Host sums per-core outputs; score[b] = main[b//256, b%256] + se[0, b].
"""
import numpy as np

SEQ, BATCH, NTAGS = 512, 512, 128
NCORES = 8
SLICE = SEQ // NCORES            # 64 s-rows per core
NROWS = SLICE * BATCH            # 32768 rows per core
NBLK = NROWS // 128              # 256 blocks of 128 rows
P = 128
XC = 16                          # blocks per emissions chunk

_RUNNER = None


# ---------------------------------------------------------------------------
# walrus workaround: this build allows only ONE sync-wait per instruction.
def _install_tile_patch():
    import bass_rust
    import concourse.mybir as mybir
    import concourse.tile as tile
    from concourse.vector_clock import ScopedClock

    if getattr(tile.TileContext, "_crf_patched", False):
        return

    def _drain_and_barrier(self, tick_clock, wait_clock):
        nc = self.nc
        drain_inst = nc.sync.drain()
        wait_clock.add_sem_waits(
            drain_inst.ins, ScopedClock({None: tick_clock.global_clock})
        )
        si = drain_inst.ins.sync_info
        waits = list(si.on_wait) if si is not None and si.on_wait else []
        if len(waits) > 1:
            si.on_wait = waits[:1]
            for w in waits[1:]:
                extra = nc.sync.drain()
                if extra.ins.sync_info is None:
                    extra.ins.sync_info = bass_rust.SyncInfo(on_wait=[], on_update=[])
                extra.ins.sync_info.on_wait = [w]
        nc.all_engine_barrier()
        assert self.sems is not None
        popped = nc._tile_sem_poison_stack.pop()
        assert popped is self._sem_poison
        nc.clear_and_free_semaphores(list(self.sems.allocated().values()))
        nc.all_engine_barrier()

    orig_commit = tile.TileContext._commit_instruction

    def _commit(self, inst, lazy_reg_writes=True):
        si = getattr(inst, "sync_info", None)
        if (
            si is not None
            and si.on_wait
            and len(si.on_wait) > 1
            and inst.engine != mybir.EngineType.Unassigned
        ):
            waits = list(si.on_wait)
            si.on_wait = waits[:1]
            for w in waits[1:]:
                nop = mybir.InstNoOp(name=f"I-{self.nc.next_id()}", ins=[], outs=[])
                nop.engine = inst.engine
                nop.sync_info = bass_rust.SyncInfo(on_wait=[w], on_update=[])
                self._add_instruction(nop)
        return orig_commit(self, inst, lazy_reg_writes)

    tile.TileContext._drain_and_barrier = _drain_and_barrier
    tile.TileContext._commit_instruction = _commit
    tile.TileContext._crf_patched = True


# ---------------------------------------------------------------------------
def _build_nc():
    import concourse.bass as bass
    import concourse.mybir as mybir
    import concourse.tile as tile

    F32, I32, BF16 = mybir.dt.float32, mybir.dt.int32, mybir.dt.bfloat16
    FP8 = mybir.dt.float8e4
    AL = mybir.AluOpType

    nc = bass.Bass()
    em = nc.declare_dram_parameter("em", [NROWS * NTAGS], BF16, isOutput=False)
    oht_d = nc.declare_dram_parameter("oht", [P * NROWS], FP8, isOutput=False)
    ttab_d = nc.declare_dram_parameter("ttab", [P * NTAGS], FP8, isOutput=False)
    tagt_d = nc.declare_dram_parameter("tagt", [P * NBLK], BF16, isOutput=False)
    mem_d = nc.declare_dram_parameter("memf", [P * NBLK], F32, isOutput=False)
    par_d = nc.declare_dram_parameter("par", [P * 2], F32, isOutput=False)
    seo_d = nc.declare_dram_parameter("seoht", [P * 1024], FP8, isOutput=False)
    sev_d = nc.declare_dram_parameter("sevals", [P * 2], FP8, isOutput=False)
    out_m = nc.declare_dram_parameter("out_m", [2, NBLK], F32, isOutput=True)
    out_se = nc.declare_dram_parameter("out_se", [1, BATCH], F32, isOutput=True)

    with tile.TileContext(nc) as tc:
        with tc.tile_pool(name="sbuf", bufs=1) as sb, \
             tc.tile_pool(name="psum", bufs=1, space="PSUM") as ps, \
             tc.tile_pool(name="emp", bufs=3) as emp:
            # ---- constants / staging
            iota_i = sb.tile([P, NTAGS], I32, name="iota_i")
            nc.gpsimd.iota(iota_i[:], pattern=[[1, NTAGS]], base=0, channel_multiplier=0)
            iota = sb.tile([P, NTAGS], BF16, name="iota")
            nc.vector.tensor_copy(out=iota[:], in_=iota_i[:])
            iop_i = sb.tile([P, 1], I32, name="iop_i")
            nc.gpsimd.iota(iop_i[:], pattern=[[0, 1]], base=0, channel_multiplier=1)
            iop = sb.tile([P, 1], F32, name="iop")
            nc.vector.tensor_copy(out=iop[:], in_=iop_i[:])
            ident = sb.tile([P, NTAGS], BF16, name="ident")
            nc.vector.tensor_scalar(out=ident[:], in0=iota[:], scalar1=iop[:],
                                    scalar2=None, op0=AL.is_equal)

            ttab = sb.tile([P, NTAGS], FP8, name="ttab")
            nc.sync.dma_start(out=ttab[:], in_=ttab_d[:].rearrange("(p t) -> p t", p=P))
            ohts = sb.tile([P, NROWS], FP8, name="ohts")
            nc.sync.dma_start(out=ohts[:], in_=oht_d[:].rearrange("(p r) -> p r", p=P))
            tagt = sb.tile([P, NBLK], BF16, name="tagt")
            nc.sync.dma_start(out=tagt[:], in_=tagt_d[:].rearrange("(p x) -> p x", p=P))
            memf = sb.tile([P, NBLK], F32, name="memf")
            nc.sync.dma_start(out=memf[:], in_=mem_d[:].rearrange("(p x) -> p x", p=P))
            par = sb.tile([P, 2], F32, name="par")
            nc.sync.dma_start(out=par[:], in_=par_d[:].rearrange("(p h) -> p h", p=P))
            seoht = sb.tile([P, 1024], FP8, name="seoht")
            nc.sync.dma_start(out=seoht[:], in_=seo_d[:].rearrange("(p b) -> p b", p=P))
            sevals = sb.tile([P, 2], FP8, name="sevals")
            nc.sync.dma_start(out=sevals[:], in_=sev_d[:].rearrange("(p c) -> p c", p=P))

            # ---- start/end terms: [1, 512] = sv[t] x oht0 + ev[t] x ohtE
            se_ps = ps.tile([1, BATCH], F32, name="se_ps", tag="pse")
            nc.tensor.matmul(out=se_ps[:], lhsT=sevals[:, 0:1],
                             rhs=seoht[:, 0:BATCH], start=True, stop=False)
            nc.tensor.matmul(out=se_ps[:], lhsT=sevals[:, 1:2],
                             rhs=seoht[:, BATCH:2 * BATCH], start=False, stop=True)
            se_sb = sb.tile([1, BATCH], F32, name="se_sb")
            nc.vector.tensor_copy(out=se_sb[:], in_=se_ps[:])
            nc.sync.dma_start(out=out_se[:], in_=se_sb[:])

            # ---- main loop
            macc = sb.tile([P, NBLK], F32, name="macc")
            junks = [sb.tile([P, NTAGS], BF16, name=f"junk{i}", tag=f"jk{i}")
                     for i in range(8)]
            tnsbs = [sb.tile([P, 4 * NTAGS], BF16, name=f"tnsb{i}", tag=f"tn{i}")
                     for i in range(4)]
            emch = None
            tn_ps = None
            for x in range(NBLK):
                d, sub = x // XC, x % XC
                if sub == 0:
                    emch = emp.tile([P, XC * NTAGS], BF16, name=f"emch{d}", tag="emch")
                    nc.sync.dma_start(
                        out=emch[:],
                        in_=em[:].rearrange("(q x t) -> q (x t)", q=P, x=NBLK)
                            [:, d * XC * NTAGS:(d + 1) * XC * NTAGS],
                    )
                g, off = x // 4, (x % 4) * NTAGS
                if x % 4 == 0:
                    tn_ps = ps.tile([P, 4 * NTAGS], F32, name=f"tnps{g % 2}",
                                    tag=f"tb{g % 2}")
                nc.tensor.matmul(
                    out=tn_ps[:, off:off + NTAGS],
                    lhsT=ohts[:, x * P:(x + 1) * P],
                    rhs=ttab[:], start=True, stop=False,
                )
                nc.tensor.matmul(
                    out=tn_ps[:, off:off + NTAGS],
                    lhsT=ident[:],
                    rhs=emch[:, sub * NTAGS:(sub + 1) * NTAGS],
                    start=False, stop=True,
                )
                if x % 4 == 3:
                    tnsb = tnsbs[g % 4]
                    nc.scalar.copy(out=tnsb[:], in_=tn_ps[:])
                    for xx in range(g * 4, g * 4 + 4):
                        o2 = (xx % 4) * NTAGS
                        nc.vector.scalar_tensor_tensor(
                            out=junks[xx % 8][:], in0=iota[:],
                            scalar=tagt[:, xx:xx + 1],
                            in1=tnsb[:, o2:o2 + NTAGS],
                            op0=AL.is_equal, op1=AL.mult,
                            accum_out=macc[:, xx:xx + 1],
                        )

            # ---- epilogue: contrib = macc*memf; parity-sum over q
            contrib = sb.tile([P, NBLK], F32, name="contrib")
            nc.vector.tensor_tensor(out=contrib[:], in0=macc[:], in1=memf[:],
                                    op=AL.mult)
            mainp = ps.tile([2, NBLK], F32, name="mainp", tag="pse")
            nc.tensor.matmul(out=mainp[:], lhsT=par[:], rhs=contrib[:],
                             start=True, stop=True)
            mains = sb.tile([2, NBLK], F32, name="mains")
            nc.vector.tensor_copy(out=mains[:], in_=mainp[:])
            nc.sync.dma_start(out=out_m[:], in_=mains[:])

    return nc


# ---------------------------------------------------------------------------
def _make_runner(nc, n_cores=8):
    import jax
    from jax.sharding import Mesh, PartitionSpec
    from jax.experimental.shard_map import shard_map
    import concourse.mybir as mybir
    from concourse import bass2jax

    bass2jax.install_neuronx_cc_hook()
    partition_name = nc.partition_id_tensor.name if nc.partition_id_tensor else None
    in_names, out_names, out_avals, zero_outs = [], [], [], []
    for alloc in nc.m.functions[0].allocations:
        if not isinstance(alloc, mybir.MemoryLocationSet):
            continue
        name = alloc.memorylocations[0].name
        if alloc.kind == "ExternalInput":
            if name != partition_name:
                in_names.append(name)
        elif alloc.kind == "ExternalOutput":
            shape = tuple(alloc.tensor_shape)
            dtype = mybir.dt.np(alloc.dtype)
            out_names.append(name)
            out_avals.append(jax.core.ShapedArray(shape, dtype))
            zero_outs.append(np.zeros(shape, dtype))
    n_params = len(in_names)
    all_in_names = list(in_names) + list(out_names)
    if partition_name is not None:
        all_in_names.append(partition_name)

    def _body(*args):
        operands = list(args)
        if partition_name is not None:
            operands.append(bass2jax.partition_id_tensor())
        outs = bass2jax._bass_exec_p.bind(
            *operands, out_avals=tuple(out_avals), in_names=tuple(all_in_names),
            out_names=tuple(out_names), lowering_input_output_aliases=(),
            sim_require_finite=True, sim_require_nnan=True, nc=nc,
        )
        return tuple(outs)

    devices = jax.devices()[:n_cores]
    mesh = Mesh(np.asarray(devices), ("core",))
    n_outs = len(out_names)
    jitted = jax.jit(
        shard_map(_body, mesh=mesh,
                  in_specs=(PartitionSpec("core"),) * (n_params + n_outs),
                  out_specs=(PartitionSpec("core"),) * n_outs, check_rep=False),
        keep_unused=True,
    )

    def run(in_maps):
        per_core = [[np.asarray(m[nm]) for nm in in_names] for m in in_maps]
        concat_in = [np.concatenate([per_core[c][i] for c in range(n_cores)], axis=0)
                     for i in range(n_params)]
        concat_zero = [np.concatenate([z] * n_cores, axis=0) for z in zero_outs]
        outs = [np.asarray(o) for o in jitted(*concat_in, *concat_zero)]
        results = []
        for c in range(n_cores):
            d = {}
            for i, nm in enumerate(out_names):
                per = outs[i].shape[0] // n_cores
                d[nm] = outs[i][c * per:(c + 1) * per]
            results.append(d)
        return results

    return run


def _get_runner():
    global _RUNNER
    if _RUNNER is None:
        _install_tile_patch()
        _RUNNER = _make_runner(_build_nc(), NCORES)
    return _RUNNER


# ---------------------------------------------------------------------------
def make_in_maps(emissions, tags, mask, start_transitions, end_transitions,
                 transitions):
    import ml_dtypes
    BF16, FP8 = ml_dtypes.bfloat16, ml_dtypes.float8_e4m3

    emissions = np.asarray(emissions, dtype=np.float32)
    tg = np.asarray(tags).astype(np.int64)
    msk = np.asarray(mask).astype(np.int64)

    ttab = np.ascontiguousarray(np.asarray(transitions, np.float32).T).astype(FP8)
    par = np.zeros((P, 2), np.float32)
    par[0::2, 0] = 1.0
    par[1::2, 1] = 1.0

    # start/end one-hot [128, 1024]: cols 0:512 onehot(tag[0,b]),
    # cols 512:1024 onehot(tags[seq_end_b, b])
    bidx = np.arange(BATCH)
    seq_end = msk.sum(axis=0).astype(np.int64) - 1
    last_tag = tg[seq_end, bidx]
    oht0 = np.zeros((P, BATCH), np.float32)
    oht0[tg[0], bidx] = 1.0
    ohtE = np.zeros((P, BATCH), np.float32)
    ohtE[last_tag, bidx] = 1.0
    zero_se = np.zeros((P, BATCH), np.float32)
    sevals = np.stack([np.asarray(start_transitions, np.float32),
                       np.asarray(end_transitions, np.float32)], axis=1)  # [128,2]
    sev8 = sevals.astype(FP8)

    # (q, x) grids: s_local = q//2, b = 256*(q&1) + x
    q = np.arange(P)[:, None]
    x = np.arange(NBLK)[None, :]
    slg = q // 2
    bg = 256 * (q & 1) + x

    in_maps = []
    for k in range(NCORES):
        s0 = k * SLICE
        sgb = np.broadcast_to(s0 + slg, (P, NBLK))
        tag = tg[sgb, bg]
        em_k = emissions[s0:s0 + SLICE].astype(BF16).reshape(-1)

        maskem = (msk[sgb, bg] != 0).astype(np.float32)
        if k == 0:
            maskem[sgb == 0] = 1.0  # reference counts emit[0] unconditionally

        # trans one-hots: oht[p, x*128 + q] = (tagnext[q,x]==p) * masktr[q,x]
        last = sgb == SEQ - 1
        snext = np.minimum(sgb + 1, SEQ - 1)
        masktr = np.where(last, 0, msk[snext, bg]).astype(np.float32)
        tagn = np.where(last, 0, tg[snext, bg])
        col = x * P + q
        oht = np.zeros((P, NROWS), dtype=np.float32)
        oht[tagn.reshape(-1), col.reshape(-1)] = masktr.reshape(-1)

        seoht = np.concatenate(
            [oht0 if k == 0 else zero_se,
             ohtE if k == NCORES - 1 else zero_se], axis=1)  # [128, 1024]

        in_maps.append({
            "em": em_k,
            "oht": oht.astype(FP8).reshape(-1),
            "ttab": ttab.reshape(-1),
            "tagt": tag.astype(BF16).reshape(-1),
            "memf": maskem.reshape(-1),
            "par": par.reshape(-1),
            "seoht": seoht.astype(FP8).reshape(-1),
            "sevals": sev8.reshape(-1),
        })
    return in_maps


def kernel(emissions, tags, mask, start_transitions, end_transitions,
           transitions):
    run = _get_runner()
    in_maps = make_in_maps(emissions, tags, mask, start_transitions,
                           end_transitions, transitions)
    results = run(in_maps)
    main = np.zeros((2, NBLK), np.float64)
    se = np.zeros((1, BATCH), np.float64)
    for r_ in results:
        main += r_["out_m"].astype(np.float64)
        se += r_["out_se"].astype(np.float64)
    score = main.reshape(BATCH) + se[0]              # b = h*256 + x
    return score.astype(np.float32)


# revision 16
# speedup vs baseline: 2.5042x; 1.1376x over previous
"""CRF sequence-score kernel for Trainium2 (8 NeuronCores, SPMD).

Strategy (S-shard: core k owns s in [64k, 64k+64), all 512 batches):
  rows r = q*256 + x laid out as [q=128 partitions, x=256 cols];
  (s_local, b) = (q//2, 256*(q&1) + x).
  Per block x, ONE PSUM tile accumulates  M[m, t] = em[row_m, t]
  + T[t, tagnext_m]*masktr_m  via two PE matmuls:
    (1) lhsT = host-built fp8 onehot(tagnext)*masktr,  rhs = T^T fp8
    (2) lhsT = identity bf16,                          rhs = em chunk bf16
  PSUM -> SBUF bf16 in 4-block groups (Act), then ONE DVE
  scalar_tensor_tensor per block selects t = tag_m:
    macc[m, x] = em[row, tag] + T[tag, tagnext]*masktr.
  contrib = macc * maskem  (exact for step masks: maskem=0 => masktr=0);
  per-b reduction over s via parity matmul -> [2, 256].
  start/end terms: TWO matmuls  startv/endv (fp8 col) x host one-hot of
  tag[0,b] / tags[seq_end,b]  -> [1, 512]  (start real on core 0 only,
  end on core 7 only).
Host sums per-core outputs; score[b] = main[b//256, b%256] + se[0, b].
"""
import numpy as np

SEQ, BATCH, NTAGS = 512, 512, 128
NCORES = 8
SLICE = SEQ // NCORES            # 64 s-rows per core
NROWS = SLICE * BATCH            # 32768 rows per core
NBLK = NROWS // 128              # 256 blocks of 128 rows
P = 128
XC = 16                          # blocks per emissions chunk

_RUNNER = None


# ---------------------------------------------------------------------------
# walrus workaround: this build allows only ONE sync-wait per instruction.
def _install_tile_patch():
    import bass_rust
    import concourse.mybir as mybir
    import concourse.tile as tile
    from concourse.vector_clock import ScopedClock

    if getattr(tile.TileContext, "_crf_patched", False):
        return

    def _drain_and_barrier(self, tick_clock, wait_clock):
        nc = self.nc
        drain_inst = nc.sync.drain()
        wait_clock.add_sem_waits(
            drain_inst.ins, ScopedClock({None: tick_clock.global_clock})
        )
        si = drain_inst.ins.sync_info
        waits = list(si.on_wait) if si is not None and si.on_wait else []
        if len(waits) > 1:
            si.on_wait = waits[:1]
            for w in waits[1:]:
                extra = nc.sync.drain()
                if extra.ins.sync_info is None:
                    extra.ins.sync_info = bass_rust.SyncInfo(on_wait=[], on_update=[])
                extra.ins.sync_info.on_wait = [w]
        nc.all_engine_barrier()
        assert self.sems is not None
        popped = nc._tile_sem_poison_stack.pop()
        assert popped is self._sem_poison
        nc.clear_and_free_semaphores(list(self.sems.allocated().values()))
        nc.all_engine_barrier()

    orig_commit = tile.TileContext._commit_instruction

    def _commit(self, inst, lazy_reg_writes=True):
        si = getattr(inst, "sync_info", None)
        if (
            si is not None
            and si.on_wait
            and len(si.on_wait) > 1
            and inst.engine != mybir.EngineType.Unassigned
        ):
            waits = list(si.on_wait)
            si.on_wait = waits[:1]
            for w in waits[1:]:
                nop = mybir.InstNoOp(name=f"I-{self.nc.next_id()}", ins=[], outs=[])
                nop.engine = inst.engine
                nop.sync_info = bass_rust.SyncInfo(on_wait=[w], on_update=[])
                self._add_instruction(nop)
        return orig_commit(self, inst, lazy_reg_writes)

    tile.TileContext._drain_and_barrier = _drain_and_barrier
    tile.TileContext._commit_instruction = _commit
    tile.TileContext._crf_patched = True


# ---------------------------------------------------------------------------
def _build_nc():
    import concourse.bass as bass
    import concourse.mybir as mybir
    import concourse.tile as tile

    F32, I32, BF16 = mybir.dt.float32, mybir.dt.int32, mybir.dt.bfloat16
    FP8 = mybir.dt.float8e4
    AL = mybir.AluOpType

    nc = bass.Bass()
    em = nc.declare_dram_parameter("em", [NROWS * NTAGS], BF16, isOutput=False)
    oht_d = nc.declare_dram_parameter("oht", [P * NROWS], FP8, isOutput=False)
    ttab_d = nc.declare_dram_parameter("ttab", [P * NTAGS], FP8, isOutput=False)
    tagt_d = nc.declare_dram_parameter("tagt", [P * NBLK], BF16, isOutput=False)
    mem_d = nc.declare_dram_parameter("memf", [P * NBLK], F32, isOutput=False)
    par_d = nc.declare_dram_parameter("par", [P * 2], F32, isOutput=False)
    seo_d = nc.declare_dram_parameter("seoht", [P * 1024], FP8, isOutput=False)
    sev_d = nc.declare_dram_parameter("sevals", [P * 2], FP8, isOutput=False)
    out_m = nc.declare_dram_parameter("out_m", [2, NBLK], F32, isOutput=True)
    out_se = nc.declare_dram_parameter("out_se", [1, BATCH], F32, isOutput=True)

    with tile.TileContext(nc) as tc:
        with tc.tile_pool(name="sbuf", bufs=1) as sb, \
             tc.tile_pool(name="psum", bufs=1, space="PSUM") as ps, \
             tc.tile_pool(name="emp", bufs=3) as emp:
            # ---- constants / staging
            iota_i = sb.tile([P, NTAGS], I32, name="iota_i")
            nc.gpsimd.iota(iota_i[:], pattern=[[1, NTAGS]], base=0, channel_multiplier=0)
            iota = sb.tile([P, NTAGS], BF16, name="iota")
            nc.vector.tensor_copy(out=iota[:], in_=iota_i[:])
            iop_i = sb.tile([P, 1], I32, name="iop_i")
            nc.gpsimd.iota(iop_i[:], pattern=[[0, 1]], base=0, channel_multiplier=1)
            iop = sb.tile([P, 1], F32, name="iop")
            nc.vector.tensor_copy(out=iop[:], in_=iop_i[:])
            ident = sb.tile([P, NTAGS], BF16, name="ident")
            nc.vector.tensor_scalar(out=ident[:], in0=iota[:], scalar1=iop[:],
                                    scalar2=None, op0=AL.is_equal)

            ttab = sb.tile([P, NTAGS], FP8, name="ttab")
            nc.sync.dma_start(out=ttab[:], in_=ttab_d[:].rearrange("(p t) -> p t", p=P))
            ohts = sb.tile([P, NROWS], FP8, name="ohts")
            tagt = sb.tile([P, NBLK], BF16, name="tagt")
            nc.sync.dma_start(out=tagt[:], in_=tagt_d[:].rearrange("(p x) -> p x", p=P))
            memf = sb.tile([P, NBLK], F32, name="memf")
            nc.sync.dma_start(out=memf[:], in_=mem_d[:].rearrange("(p x) -> p x", p=P))
            par = sb.tile([P, 2], F32, name="par")
            nc.sync.dma_start(out=par[:], in_=par_d[:].rearrange("(p h) -> p h", p=P))
            seoht = sb.tile([P, 1024], FP8, name="seoht")
            nc.sync.dma_start(out=seoht[:], in_=seo_d[:].rearrange("(p b) -> p b", p=P))
            sevals = sb.tile([P, 2], FP8, name="sevals")
            nc.sync.dma_start(out=sevals[:], in_=sev_d[:].rearrange("(p c) -> p c", p=P))

            # ---- main loop
            macc = sb.tile([P, NBLK], F32, name="macc")
            junks = [sb.tile([P, NTAGS], BF16, name=f"junk{i}", tag=f"jk{i}")
                     for i in range(8)]
            pjunks = [sb.tile([P, NTAGS], BF16, name=f"pjunk{i}", tag=f"pj{i}")
                      for i in range(4)]
            tnsbs = [sb.tile([P, 4 * NTAGS], BF16, name=f"tnsb{i}", tag=f"tn{i}")
                     for i in range(4)]
            emch = None
            tn_ps = None
            for x in range(NBLK):
                d, sub = x // XC, x % XC
                if sub == 0:
                    nc.sync.dma_start(
                        out=ohts[:, d * XC * P:(d + 1) * XC * P],
                        in_=oht_d[:].rearrange("(q r) -> q r", q=P)
                            [:, d * XC * P:(d + 1) * XC * P],
                    )
                    emch = emp.tile([P, XC * NTAGS], BF16, name=f"emch{d}", tag="emch")
                    nc.sync.dma_start(
                        out=emch[:],
                        in_=em[:].rearrange("(q x t) -> q (x t)", q=P, x=NBLK)
                            [:, d * XC * NTAGS:(d + 1) * XC * NTAGS],
                    )
                g, off = x // 4, (x % 4) * NTAGS
                if x % 4 == 0:
                    tn_ps = ps.tile([P, 4 * NTAGS], F32, name=f"tnps{g % 2}",
                                    tag=f"tb{g % 2}")
                nc.tensor.matmul(
                    out=tn_ps[:, off:off + NTAGS],
                    lhsT=ohts[:, x * P:(x + 1) * P],
                    rhs=ttab[:], start=True, stop=False,
                )
                nc.tensor.matmul(
                    out=tn_ps[:, off:off + NTAGS],
                    lhsT=ident[:],
                    rhs=emch[:, sub * NTAGS:(sub + 1) * NTAGS],
                    start=False, stop=True,
                )
                if x % 4 == 3:
                    tnsb = tnsbs[g % 4]
                    nc.scalar.copy(out=tnsb[:], in_=tn_ps[:])
                    for xx in range(g * 4, g * 4 + 4):
                        o2 = (xx % 4) * NTAGS
                        nc.vector.scalar_tensor_tensor(
                            out=junks[xx % 8][:], in0=iota[:],
                            scalar=tagt[:, xx:xx + 1],
                            in1=tnsb[:, o2:o2 + NTAGS],
                            op0=AL.is_equal, op1=AL.mult,
                            accum_out=macc[:, xx:xx + 1],
                        )

            # ---- start/end terms: [1, 512] = sv[t] x oht0 + ev[t] x ohtE
            se_ps = ps.tile([1, BATCH], F32, name="se_ps", tag="pse")
            nc.tensor.matmul(out=se_ps[:], lhsT=sevals[:, 0:1],
                             rhs=seoht[:, 0:BATCH], start=True, stop=False)
            nc.tensor.matmul(out=se_ps[:], lhsT=sevals[:, 1:2],
                             rhs=seoht[:, BATCH:2 * BATCH], start=False, stop=True)
            se_sb = sb.tile([1, BATCH], F32, name="se_sb")
            nc.vector.tensor_copy(out=se_sb[:], in_=se_ps[:])
            nc.sync.dma_start(out=out_se[:], in_=se_sb[:])

            # ---- epilogue: contrib = macc*memf; parity-sum over q
            contrib = sb.tile([P, NBLK], F32, name="contrib")
            nc.vector.tensor_tensor(out=contrib[:], in0=macc[:], in1=memf[:],
                                    op=AL.mult)
            mainp = ps.tile([2, NBLK], F32, name="mainp", tag="pse")
            nc.tensor.matmul(out=mainp[:], lhsT=par[:], rhs=contrib[:],
                             start=True, stop=True)
            mains = sb.tile([2, NBLK], F32, name="mains")
            nc.vector.tensor_copy(out=mains[:], in_=mainp[:])
            nc.sync.dma_start(out=out_m[:], in_=mains[:])

    return nc


# ---------------------------------------------------------------------------
def _make_runner(nc, n_cores=8):
    import jax
    from jax.sharding import Mesh, PartitionSpec
    from jax.experimental.shard_map import shard_map
    import concourse.mybir as mybir
    from concourse import bass2jax

    bass2jax.install_neuronx_cc_hook()
    partition_name = nc.partition_id_tensor.name if nc.partition_id_tensor else None
    in_names, out_names, out_avals, zero_outs = [], [], [], []
    for alloc in nc.m.functions[0].allocations:
        if not isinstance(alloc, mybir.MemoryLocationSet):
            continue
        name = alloc.memorylocations[0].name
        if alloc.kind == "ExternalInput":
            if name != partition_name:
                in_names.append(name)
        elif alloc.kind == "ExternalOutput":
            shape = tuple(alloc.tensor_shape)
            dtype = mybir.dt.np(alloc.dtype)
            out_names.append(name)
            out_avals.append(jax.core.ShapedArray(shape, dtype))
            zero_outs.append(np.zeros(shape, dtype))
    n_params = len(in_names)
    all_in_names = list(in_names) + list(out_names)
    if partition_name is not None:
        all_in_names.append(partition_name)

    def _body(*args):
        operands = list(args)
        if partition_name is not None:
            operands.append(bass2jax.partition_id_tensor())
        outs = bass2jax._bass_exec_p.bind(
            *operands, out_avals=tuple(out_avals), in_names=tuple(all_in_names),
            out_names=tuple(out_names), lowering_input_output_aliases=(),
            sim_require_finite=True, sim_require_nnan=True, nc=nc,
        )
        return tuple(outs)

    devices = jax.devices()[:n_cores]
    mesh = Mesh(np.asarray(devices), ("core",))
    n_outs = len(out_names)
    jitted = jax.jit(
        shard_map(_body, mesh=mesh,
                  in_specs=(PartitionSpec("core"),) * (n_params + n_outs),
                  out_specs=(PartitionSpec("core"),) * n_outs, check_rep=False),
        keep_unused=True,
    )

    def run(in_maps):
        per_core = [[np.asarray(m[nm]) for nm in in_names] for m in in_maps]
        concat_in = [np.concatenate([per_core[c][i] for c in range(n_cores)], axis=0)
                     for i in range(n_params)]
        concat_zero = [np.concatenate([z] * n_cores, axis=0) for z in zero_outs]
        outs = [np.asarray(o) for o in jitted(*concat_in, *concat_zero)]
        results = []
        for c in range(n_cores):
            d = {}
            for i, nm in enumerate(out_names):
                per = outs[i].shape[0] // n_cores
                d[nm] = outs[i][c * per:(c + 1) * per]
            results.append(d)
        return results

    return run


def _get_runner():
    global _RUNNER
    if _RUNNER is None:
        _install_tile_patch()
        _RUNNER = _make_runner(_build_nc(), NCORES)
    return _RUNNER


# ---------------------------------------------------------------------------
def make_in_maps(emissions, tags, mask, start_transitions, end_transitions,
                 transitions):
    import ml_dtypes
    BF16, FP8 = ml_dtypes.bfloat16, ml_dtypes.float8_e4m3

    emissions = np.asarray(emissions, dtype=np.float32)
    tg = np.asarray(tags).astype(np.int64)
    msk = np.asarray(mask).astype(np.int64)

    ttab = np.ascontiguousarray(np.asarray(transitions, np.float32).T).astype(FP8)
    par = np.zeros((P, 2), np.float32)
    par[0::2, 0] = 1.0
    par[1::2, 1] = 1.0

    # start/end one-hot [128, 1024]: cols 0:512 onehot(tag[0,b]),
    # cols 512:1024 onehot(tags[seq_end_b, b])
    bidx = np.arange(BATCH)
    seq_end = msk.sum(axis=0).astype(np.int64) - 1
    last_tag = tg[seq_end, bidx]
    oht0 = np.zeros((P, BATCH), np.float32)
    oht0[tg[0], bidx] = 1.0
    ohtE = np.zeros((P, BATCH), np.float32)
    ohtE[last_tag, bidx] = 1.0
    zero_se = np.zeros((P, BATCH), np.float32)
    sevals = np.stack([np.asarray(start_transitions, np.float32),
                       np.asarray(end_transitions, np.float32)], axis=1)  # [128,2]
    sev8 = sevals.astype(FP8)

    # (q, x) grids: s_local = q//2, b = 256*(q&1) + x
    q = np.arange(P)[:, None]
    x = np.arange(NBLK)[None, :]
    slg = q // 2
    bg = 256 * (q & 1) + x

    in_maps = []
    for k in range(NCORES):
        s0 = k * SLICE
        sgb = np.broadcast_to(s0 + slg, (P, NBLK))
        tag = tg[sgb, bg]
        em_k = emissions[s0:s0 + SLICE].astype(BF16).reshape(-1)

        maskem = (msk[sgb, bg] != 0).astype(np.float32)
        if k == 0:
            maskem[sgb == 0] = 1.0  # reference counts emit[0] unconditionally

        # trans one-hots: oht[p, x*128 + q] = (tagnext[q,x]==p) * masktr[q,x]
        last = sgb == SEQ - 1
        snext = np.minimum(sgb + 1, SEQ - 1)
        masktr = np.where(last, 0, msk[snext, bg]).astype(np.float32)
        tagn = np.where(last, 0, tg[snext, bg])
        col = x * P + q
        oht = np.zeros((P, NROWS), dtype=np.float32)
        oht[tagn.reshape(-1), col.reshape(-1)] = masktr.reshape(-1)

        seoht = np.concatenate(
            [oht0 if k == 0 else zero_se,
             ohtE if k == NCORES - 1 else zero_se], axis=1)  # [128, 1024]

        in_maps.append({
            "em": em_k,
            "oht": oht.astype(FP8).reshape(-1),
            "ttab": ttab.reshape(-1),
            "tagt": tag.astype(BF16).reshape(-1),
            "memf": maskem.reshape(-1),
            "par": par.reshape(-1),
            "seoht": seoht.astype(FP8).reshape(-1),
            "sevals": sev8.reshape(-1),
        })
    return in_maps


def kernel(emissions, tags, mask, start_transitions, end_transitions,
           transitions):
    run = _get_runner()
    in_maps = make_in_maps(emissions, tags, mask, start_transitions,
                           end_transitions, transitions)
    results = run(in_maps)
    main = np.zeros((2, NBLK), np.float64)
    se = np.zeros((1, BATCH), np.float64)
    for r_ in results:
        main += r_["out_m"].astype(np.float64)
        se += r_["out_se"].astype(np.float64)
    score = main.reshape(BATCH) + se[0]              # b = h*256 + x
    return score.astype(np.float32)


# revision 17
# speedup vs baseline: 2.5911x; 1.0347x over previous
"""CRF sequence-score kernel for Trainium2 (8 NeuronCores, SPMD).

Strategy (S-shard: core k owns s in [64k, 64k+64), all 512 batches):
  rows r = q*256 + x laid out as [q=128 partitions, x=256 cols];
  (s_local, b) = (q//2, 256*(q&1) + x).
  Per block x, ONE PSUM tile accumulates  M[m, t] = em[row_m, t]
  + T[t, tagnext_m]*masktr_m  via two PE matmuls:
    (1) lhsT = host-built fp8 onehot(tagnext)*masktr,  rhs = T^T fp8
    (2) lhsT = identity bf16,                          rhs = em chunk bf16
  PSUM -> SBUF bf16 in 4-block groups (Act), then ONE DVE
  scalar_tensor_tensor per block selects t = tag_m:
    macc[m, x] = em[row, tag] + T[tag, tagnext]*masktr.
  contrib = macc * maskem  (exact for step masks: maskem=0 => masktr=0);
  per-b reduction over s via parity matmul -> [2, 256].
  start/end terms: TWO matmuls  startv/endv (fp8 col) x host one-hot of
  tag[0,b] / tags[seq_end,b]  -> [1, 512]  (start real on core 0 only,
  end on core 7 only).
Host sums per-core outputs; score[b] = main[b//256, b%256] + se[0, b].
"""
import numpy as np

SEQ, BATCH, NTAGS = 512, 512, 128
NCORES = 8
SLICE = SEQ // NCORES            # 64 s-rows per core
NROWS = SLICE * BATCH            # 32768 rows per core
NBLK = NROWS // 128              # 256 blocks of 128 rows
P = 128
XC = 16                          # blocks per emissions chunk

_RUNNER = None


# ---------------------------------------------------------------------------
# walrus workaround: this build allows only ONE sync-wait per instruction.
def _install_tile_patch():
    import bass_rust
    import concourse.mybir as mybir
    import concourse.tile as tile
    from concourse.vector_clock import ScopedClock

    if getattr(tile.TileContext, "_crf_patched", False):
        return

    def _drain_and_barrier(self, tick_clock, wait_clock):
        nc = self.nc
        drain_inst = nc.sync.drain()
        wait_clock.add_sem_waits(
            drain_inst.ins, ScopedClock({None: tick_clock.global_clock})
        )
        si = drain_inst.ins.sync_info
        waits = list(si.on_wait) if si is not None and si.on_wait else []
        if len(waits) > 1:
            si.on_wait = waits[:1]
            for w in waits[1:]:
                extra = nc.sync.drain()
                if extra.ins.sync_info is None:
                    extra.ins.sync_info = bass_rust.SyncInfo(on_wait=[], on_update=[])
                extra.ins.sync_info.on_wait = [w]
        nc.all_engine_barrier()
        assert self.sems is not None
        popped = nc._tile_sem_poison_stack.pop()
        assert popped is self._sem_poison
        nc.clear_and_free_semaphores(list(self.sems.allocated().values()))
        nc.all_engine_barrier()

    orig_commit = tile.TileContext._commit_instruction

    def _commit(self, inst, lazy_reg_writes=True):
        si = getattr(inst, "sync_info", None)
        if (
            si is not None
            and si.on_wait
            and len(si.on_wait) > 1
            and inst.engine != mybir.EngineType.Unassigned
        ):
            waits = list(si.on_wait)
            si.on_wait = waits[:1]
            for w in waits[1:]:
                nop = mybir.InstNoOp(name=f"I-{self.nc.next_id()}", ins=[], outs=[])
                nop.engine = inst.engine
                nop.sync_info = bass_rust.SyncInfo(on_wait=[w], on_update=[])
                self._add_instruction(nop)
        return orig_commit(self, inst, lazy_reg_writes)

    tile.TileContext._drain_and_barrier = _drain_and_barrier
    tile.TileContext._commit_instruction = _commit
    tile.TileContext._crf_patched = True


# ---------------------------------------------------------------------------
def _build_nc():
    import concourse.bass as bass
    import concourse.mybir as mybir
    import concourse.tile as tile

    F32, I32, BF16 = mybir.dt.float32, mybir.dt.int32, mybir.dt.bfloat16
    FP8 = mybir.dt.float8e4
    AL = mybir.AluOpType

    nc = bass.Bass()
    em = nc.declare_dram_parameter("em", [NROWS * NTAGS], BF16, isOutput=False)
    oht_d = nc.declare_dram_parameter("oht", [P * NROWS], FP8, isOutput=False)
    ttab_d = nc.declare_dram_parameter("ttab", [P * NTAGS], FP8, isOutput=False)
    tagt_d = nc.declare_dram_parameter("tagt", [P * NBLK], BF16, isOutput=False)
    mem_d = nc.declare_dram_parameter("memf", [P * NBLK], F32, isOutput=False)
    par_d = nc.declare_dram_parameter("par", [P * 2], F32, isOutput=False)
    seo_d = nc.declare_dram_parameter("seoht", [P * 1024], FP8, isOutput=False)
    sev_d = nc.declare_dram_parameter("sevals", [P * 2], FP8, isOutput=False)
    out_m = nc.declare_dram_parameter("out_m", [2, NBLK], F32, isOutput=True)
    out_se = nc.declare_dram_parameter("out_se", [1, BATCH], F32, isOutput=True)

    with tile.TileContext(nc) as tc:
        with tc.tile_pool(name="sbuf", bufs=1) as sb, \
             tc.tile_pool(name="psum", bufs=1, space="PSUM") as ps, \
             tc.tile_pool(name="emp", bufs=3) as emp:
            # ---- constants / staging
            iota_i = sb.tile([P, NTAGS], I32, name="iota_i")
            nc.gpsimd.iota(iota_i[:], pattern=[[1, NTAGS]], base=0, channel_multiplier=0)
            iota = sb.tile([P, NTAGS], BF16, name="iota")
            nc.vector.tensor_copy(out=iota[:], in_=iota_i[:])
            iop_i = sb.tile([P, 1], I32, name="iop_i")
            nc.gpsimd.iota(iop_i[:], pattern=[[0, 1]], base=0, channel_multiplier=1)
            iop = sb.tile([P, 1], F32, name="iop")
            nc.vector.tensor_copy(out=iop[:], in_=iop_i[:])
            ident = sb.tile([P, NTAGS], BF16, name="ident")
            nc.vector.tensor_scalar(out=ident[:], in0=iota[:], scalar1=iop[:],
                                    scalar2=None, op0=AL.is_equal)

            ttab = sb.tile([P, NTAGS], FP8, name="ttab")
            nc.sync.dma_start(out=ttab[:], in_=ttab_d[:].rearrange("(p t) -> p t", p=P))
            ohts = sb.tile([P, NROWS], FP8, name="ohts")
            tagt = sb.tile([P, NBLK], BF16, name="tagt")
            nc.sync.dma_start(out=tagt[:], in_=tagt_d[:].rearrange("(p x) -> p x", p=P))
            memf = sb.tile([P, NBLK], F32, name="memf")
            par = sb.tile([P, 2], F32, name="par")
            seoht = sb.tile([P, 1024], FP8, name="seoht")
            sevals = sb.tile([P, 2], FP8, name="sevals")

            # ---- main loop (stt's run one group behind the Act copy)
            macc = sb.tile([P, NBLK], F32, name="macc")
            junks = [sb.tile([P, NTAGS], BF16, name=f"junk{i}", tag=f"jk{i}")
                     for i in range(8)]
            tnsbs = [sb.tile([P, 4 * NTAGS], BF16, name=f"tnsb{i}", tag=f"tn{i}")
                     for i in range(6)]
            emch = None
            tn_ps = None

            def stt_group(g):
                tnsb = tnsbs[g % 6]
                for xx in range(g * 4, g * 4 + 4):
                    o2 = (xx % 4) * NTAGS
                    nc.vector.scalar_tensor_tensor(
                        out=junks[xx % 8][:], in0=iota[:],
                        scalar=tagt[:, xx:xx + 1],
                        in1=tnsb[:, o2:o2 + NTAGS],
                        op0=AL.is_equal, op1=AL.mult,
                        accum_out=macc[:, xx:xx + 1],
                    )

            for x in range(NBLK):
                d, sub = x // XC, x % XC
                if sub == 0:
                    nc.sync.dma_start(
                        out=ohts[:, d * XC * P:(d + 1) * XC * P],
                        in_=oht_d[:].rearrange("(q r) -> q r", q=P)
                            [:, d * XC * P:(d + 1) * XC * P],
                    )
                    emch = emp.tile([P, XC * NTAGS], BF16, name=f"emch{d}", tag="emch")
                    nc.sync.dma_start(
                        out=emch[:],
                        in_=em[:].rearrange("(q x t) -> q (x t)", q=P, x=NBLK)
                            [:, d * XC * NTAGS:(d + 1) * XC * NTAGS],
                    )
                g, off = x // 4, (x % 4) * NTAGS
                if x % 4 == 0:
                    tn_ps = ps.tile([P, 4 * NTAGS], F32, name=f"tnps{g % 4}",
                                    tag=f"tb{g % 4}")
                nc.tensor.matmul(
                    out=tn_ps[:, off:off + NTAGS],
                    lhsT=ohts[:, x * P:(x + 1) * P],
                    rhs=ttab[:], start=True, stop=False,
                )
                nc.tensor.matmul(
                    out=tn_ps[:, off:off + NTAGS],
                    lhsT=ident[:],
                    rhs=emch[:, sub * NTAGS:(sub + 1) * NTAGS],
                    start=False, stop=True,
                )
                if x % 4 == 3:
                    nc.scalar.copy(out=tnsbs[g % 6][:], in_=tn_ps[:])
                    if g > 0:
                        stt_group(g - 1)
            stt_group(NBLK // 4 - 1)

            nc.sync.dma_start(out=memf[:], in_=mem_d[:].rearrange("(p x) -> p x", p=P))
            nc.sync.dma_start(out=par[:], in_=par_d[:].rearrange("(p h) -> p h", p=P))
            nc.sync.dma_start(out=seoht[:], in_=seo_d[:].rearrange("(p b) -> p b", p=P))
            nc.sync.dma_start(out=sevals[:], in_=sev_d[:].rearrange("(p c) -> p c", p=P))

            # ---- start/end terms: [1, 512] = sv[t] x oht0 + ev[t] x ohtE
            se_ps = ps.tile([1, BATCH], F32, name="se_ps", tag="pse")
            nc.tensor.matmul(out=se_ps[:], lhsT=sevals[:, 0:1],
                             rhs=seoht[:, 0:BATCH], start=True, stop=False)
            nc.tensor.matmul(out=se_ps[:], lhsT=sevals[:, 1:2],
                             rhs=seoht[:, BATCH:2 * BATCH], start=False, stop=True)
            se_sb = sb.tile([1, BATCH], F32, name="se_sb")
            nc.vector.tensor_copy(out=se_sb[:], in_=se_ps[:])
            nc.sync.dma_start(out=out_se[:], in_=se_sb[:])

            # ---- epilogue: contrib = macc*memf; parity-sum over q
            contrib = sb.tile([P, NBLK], F32, name="contrib")
            nc.vector.tensor_tensor(out=contrib[:], in0=macc[:], in1=memf[:],
                                    op=AL.mult)
            mainp = ps.tile([2, NBLK], F32, name="mainp", tag="pse")
            nc.tensor.matmul(out=mainp[:], lhsT=par[:], rhs=contrib[:],
                             start=True, stop=True)
            mains = sb.tile([2, NBLK], F32, name="mains")
            nc.vector.tensor_copy(out=mains[:], in_=mainp[:])
            nc.sync.dma_start(out=out_m[:], in_=mains[:])

    return nc


# ---------------------------------------------------------------------------
def _make_runner(nc, n_cores=8):
    import jax
    from jax.sharding import Mesh, PartitionSpec
    from jax.experimental.shard_map import shard_map
    import concourse.mybir as mybir
    from concourse import bass2jax

    bass2jax.install_neuronx_cc_hook()
    partition_name = nc.partition_id_tensor.name if nc.partition_id_tensor else None
    in_names, out_names, out_avals, zero_outs = [], [], [], []
    for alloc in nc.m.functions[0].allocations:
        if not isinstance(alloc, mybir.MemoryLocationSet):
            continue
        name = alloc.memorylocations[0].name
        if alloc.kind == "ExternalInput":
            if name != partition_name:
                in_names.append(name)
        elif alloc.kind == "ExternalOutput":
            shape = tuple(alloc.tensor_shape)
            dtype = mybir.dt.np(alloc.dtype)
            out_names.append(name)
            out_avals.append(jax.core.ShapedArray(shape, dtype))
            zero_outs.append(np.zeros(shape, dtype))
    n_params = len(in_names)
    all_in_names = list(in_names) + list(out_names)
    if partition_name is not None:
        all_in_names.append(partition_name)

    def _body(*args):
        operands = list(args)
        if partition_name is not None:
            operands.append(bass2jax.partition_id_tensor())
        outs = bass2jax._bass_exec_p.bind(
            *operands, out_avals=tuple(out_avals), in_names=tuple(all_in_names),
            out_names=tuple(out_names), lowering_input_output_aliases=(),
            sim_require_finite=True, sim_require_nnan=True, nc=nc,
        )
        return tuple(outs)

    devices = jax.devices()[:n_cores]
    mesh = Mesh(np.asarray(devices), ("core",))
    n_outs = len(out_names)
    jitted = jax.jit(
        shard_map(_body, mesh=mesh,
                  in_specs=(PartitionSpec("core"),) * (n_params + n_outs),
                  out_specs=(PartitionSpec("core"),) * n_outs, check_rep=False),
        keep_unused=True,
    )

    def run(in_maps):
        per_core = [[np.asarray(m[nm]) for nm in in_names] for m in in_maps]
        concat_in = [np.concatenate([per_core[c][i] for c in range(n_cores)], axis=0)
                     for i in range(n_params)]
        concat_zero = [np.concatenate([z] * n_cores, axis=0) for z in zero_outs]
        outs = [np.asarray(o) for o in jitted(*concat_in, *concat_zero)]
        results = []
        for c in range(n_cores):
            d = {}
            for i, nm in enumerate(out_names):
                per = outs[i].shape[0] // n_cores
                d[nm] = outs[i][c * per:(c + 1) * per]
            results.append(d)
        return results

    return run


def _get_runner():
    global _RUNNER
    if _RUNNER is None:
        _install_tile_patch()
        _RUNNER = _make_runner(_build_nc(), NCORES)
    return _RUNNER


# ---------------------------------------------------------------------------
def make_in_maps(emissions, tags, mask, start_transitions, end_transitions,
                 transitions):
    import ml_dtypes
    BF16, FP8 = ml_dtypes.bfloat16, ml_dtypes.float8_e4m3

    emissions = np.asarray(emissions, dtype=np.float32)
    tg = np.asarray(tags).astype(np.int64)
    msk = np.asarray(mask).astype(np.int64)

    ttab = np.ascontiguousarray(np.asarray(transitions, np.float32).T).astype(FP8)
    par = np.zeros((P, 2), np.float32)
    par[0::2, 0] = 1.0
    par[1::2, 1] = 1.0

    # start/end one-hot [128, 1024]: cols 0:512 onehot(tag[0,b]),
    # cols 512:1024 onehot(tags[seq_end_b, b])
    bidx = np.arange(BATCH)
    seq_end = msk.sum(axis=0).astype(np.int64) - 1
    last_tag = tg[seq_end, bidx]
    oht0 = np.zeros((P, BATCH), np.float32)
    oht0[tg[0], bidx] = 1.0
    ohtE = np.zeros((P, BATCH), np.float32)
    ohtE[last_tag, bidx] = 1.0
    zero_se = np.zeros((P, BATCH), np.float32)
    sevals = np.stack([np.asarray(start_transitions, np.float32),
                       np.asarray(end_transitions, np.float32)], axis=1)  # [128,2]
    sev8 = sevals.astype(FP8)

    # (q, x) grids: s_local = q//2, b = 256*(q&1) + x
    q = np.arange(P)[:, None]
    x = np.arange(NBLK)[None, :]
    slg = q // 2
    bg = 256 * (q & 1) + x

    in_maps = []
    for k in range(NCORES):
        s0 = k * SLICE
        sgb = np.broadcast_to(s0 + slg, (P, NBLK))
        tag = tg[sgb, bg]
        em_k = emissions[s0:s0 + SLICE].astype(BF16).reshape(-1)

        maskem = (msk[sgb, bg] != 0).astype(np.float32)
        if k == 0:
            maskem[sgb == 0] = 1.0  # reference counts emit[0] unconditionally

        # trans one-hots: oht[p, x*128 + q] = (tagnext[q,x]==p) * masktr[q,x]
        last = sgb == SEQ - 1
        snext = np.minimum(sgb + 1, SEQ - 1)
        masktr = np.where(last, 0, msk[snext, bg]).astype(np.float32)
        tagn = np.where(last, 0, tg[snext, bg])
        col = x * P + q
        oht = np.zeros((P, NROWS), dtype=np.float32)
        oht[tagn.reshape(-1), col.reshape(-1)] = masktr.reshape(-1)

        seoht = np.concatenate(
            [oht0 if k == 0 else zero_se,
             ohtE if k == NCORES - 1 else zero_se], axis=1)  # [128, 1024]

        in_maps.append({
            "em": em_k,
            "oht": oht.astype(FP8).reshape(-1),
            "ttab": ttab.reshape(-1),
            "tagt": tag.astype(BF16).reshape(-1),
            "memf": maskem.reshape(-1),
            "par": par.reshape(-1),
            "seoht": seoht.astype(FP8).reshape(-1),
            "sevals": sev8.reshape(-1),
        })
    return in_maps


def kernel(emissions, tags, mask, start_transitions, end_transitions,
           transitions):
    run = _get_runner()
    in_maps = make_in_maps(emissions, tags, mask, start_transitions,
                           end_transitions, transitions)
    results = run(in_maps)
    main = np.zeros((2, NBLK), np.float64)
    se = np.zeros((1, BATCH), np.float64)
    for r_ in results:
        main += r_["out_m"].astype(np.float64)
        se += r_["out_se"].astype(np.float64)
    score = main.reshape(BATCH) + se[0]              # b = h*256 + x
    return score.astype(np.float32)
